# revision 26
# baseline (speedup 1.0000x reference)
"""Trainium2 Bass kernel for the HNX scatter-memory block.

Sharding: 8 cores = (batch b in 0..3) x (sequence half j in 0..1).
Each core processes its 1024-token window plus W warmup tokens on each
side (zero-padded + masked at sequence edges), so both the forward and
backward EMA scans converge to the exact state before the window starts
(truncation error ~ sigmoid(decay)^W).  No inter-core communication.

On-chip layout is "scan layout": channels on partitions, time along the
free dimension.  x is pre-transposed on the host.  The EMA recurrences
use the native DVE tensor_tensor_scan.  Channel-dim reductions
(softmax-entropy, slot logits) are PE matmuls against ones / weight
columns; per-token scalars are re-broadcast across partitions with
gpsimd.partition_broadcast.

Matmuls run in bf16 (operands converted on host / written bf16 by the
producing engines; PSUM accumulation stays fp32).  The backward strand
is interleaved into the phase-1 group loop and the two phase-3 window
chunks are software-pipelined (both reduction chains issue back-to-back
on the PE) so the tensor engine never idles long enough to lose its
pstate.  b_out is added on the host; out-proj results DMA straight from
PSUM.
"""

import numpy as np
from contextlib import ExitStack

import concourse.bacc as bacc
import concourse.tile as tile
from concourse import mybir
from concourse.bass_utils import run_bass_kernel_spmd

F32 = mybir.dt.float32
F32R = mybir.dt.float32r
BF16 = mybir.dt.bfloat16
AF = mybir.ActivationFunctionType
OP = mybir.AluOpType


class Cfg:
    def __init__(self, DI=1024, H=1024, O=1024, S=128, T=2048, W=128, CH=512,
                 mm_dtype=BF16, sim_acts=False):
        self.DI, self.H, self.O, self.S, self.T, self.W, self.CH = DI, H, O, S, T, W, CH
        self.Tout = T // 2            # tokens per core window
        self.Tw = self.Tout + 2 * W   # work tokens per core
        self.KG = DI // 128           # input k-tiles
        self.HG = H // 128            # hidden channel groups
        self.OG = O // 128            # output channel groups
        # phase-1 chunks (last one may be short)
        self.chunks = [(s, min(CH, self.Tw - s)) for s in range(0, self.Tw, CH)]
        self.WCH = self.Tout // CH    # phase-3 (window) chunks
        self.mm_dtype = mm_dtype
        self.sim_acts = sim_acts
        assert self.S == 128 and self.Tout % CH == 0
        assert CH <= 512 and self.W <= CH and self.W <= self.chunks[0][1]


# chp column layout: per-channel params, one column per (param, group)
CHP_NAMES = ["k0", "k1", "omdf", "df", "omdb", "db", "sbias", "ba"]
SC_F1, SC_F0, SC_NF1, SC_F2, SC_SW, SC_SB, SC_BMG = range(7)


def build_program(cfg: Cfg):
    c = cfg
    nc = bacc.Bacc("TRN2", target_bir_lowering=False, debug=False,
                   enable_asserts=False)

    MD = c.mm_dtype  # dtype for every matmul operand (tiles + dram)

    def rd(ap):
        # non-matmul engines can read bf16 natively but not float32r
        return ap.bitcast(F32) if MD == F32R else ap

    xt = nc.dram_tensor("xt", [c.DI, c.Tw], MD, kind="ExternalInput").ap()
    w_in = nc.dram_tensor("w_in", [128, 2 * c.HG * c.KG * 128], MD,
                          kind="ExternalInput").ap()
    w_out = nc.dram_tensor("w_out", [c.H, c.O], MD, kind="ExternalInput").ap()
    w_slot = nc.dram_tensor("w_slot", [c.H, c.S], MD, kind="ExternalInput").ap()
    w_mg = nc.dram_tensor("w_mg", [c.H, 1], MD, kind="ExternalInput").ap()
    mem_bank = nc.dram_tensor("mem_bank", [c.S, c.H], MD, kind="ExternalInput").ap()
    ones_d = nc.dram_tensor("ones", [128, 1], MD, kind="ExternalInput").ap()
    bout_d = nc.dram_tensor("bout", [128, c.OG], F32, kind="ExternalInput").ap()
    chp = nc.dram_tensor("chp", [128, len(CHP_NAMES) * c.HG], F32,
                         kind="ExternalInput").ap()
    bslot_d = nc.dram_tensor("bslot", [128, 1], F32, kind="ExternalInput").ap()
    sc = nc.dram_tensor("sc", [1, 8], F32, kind="ExternalInput").ap()
    mask_d = nc.dram_tensor("mask", [1, c.Tw], F32, kind="ExternalInput").ap()
    out_d = nc.dram_tensor("outT", [c.O, c.Tout], F32, kind="ExternalOutput").ap()

    with tile.TileContext(nc) as tc:
        with ExitStack() as top:
            consts = top.enter_context(tc.tile_pool(name="consts", bufs=1))
            chp_t = consts.tile([128, len(CHP_NAMES) * c.HG], F32)
            nc.scalar.dma_start(chp_t[:], chp[:])
            sc_t = consts.tile([1, 8], F32)
            nc.scalar.dma_start(sc_t[:], sc[:])
            bout_t = consts.tile([128, c.OG], F32)
            nc.scalar.dma_start(bout_t[:], bout_d[:])
            bslot_t = consts.tile([128, 1], F32)
            nc.scalar.dma_start(bslot_t[:], bslot_d[:])
            ones_t = consts.tile([128, 1], MD)
            nc.scalar.dma_start(ones_t[:], ones_d[:])
            mb = consts.tile([128, c.Tw], F32)
            nc.scalar.dma_start(mb[:], mask_d.broadcast_to([128, c.Tw]))
            wslot_t = [consts.tile([128, c.S], MD, name=f"ws{k}", tag=f"ws{k}")
                       for k in range(c.HG)]
            for k in range(c.HG):
                nc.scalar.dma_start(wslot_t[k][:], w_slot[k * 128:(k + 1) * 128, :])
            wmg_t = [consts.tile([128, 1], MD, name=f"wmg{k}", tag=f"wmg{k}")
                     for k in range(c.HG)]
            for k in range(c.HG):
                nc.scalar.dma_start(wmg_t[k][:], w_mg[k * 128:(k + 1) * 128, :])
            memb_t = consts.tile([128, c.H], MD)
            nc.scalar.dma_start(memb_t[:], mem_bank[:])

            def chpc(name, g):
                i = CHP_NAMES.index(name) * c.HG + g
                return chp_t[:, i:i + 1]

            def scc(i):
                return sc_t[0:1, i:i + 1]

            fpool = top.enter_context(tc.tile_pool(name="f", bufs=1))
            f_t = [fpool.tile([128, c.Tw], MD, name=f"f{g}", tag=f"f{g}")
                   for g in range(c.HG)]
            gpool = top.enter_context(tc.tile_pool(name="gb", bufs=1))
            gb_t = [gpool.tile([128, c.Tw - c.W], F32, name=f"gb{g}", tag=f"gb{g}")
                    for g in range(c.HG)]
            p2s = top.enter_context(tc.tile_pool(name="p2s", bufs=2))
            psS = top.enter_context(tc.tile_pool(name="psS", bufs=1,
                                                 space="PSUM"))
            psR = top.enter_context(tc.tile_pool(name="psR", bufs=1,
                                                 space="PSUM"))
            pRs = [psR.tile([65, c.CH], F32, name=f"pR{w}", tag=f"pR{w}")
                   for w in range(c.WCH)]
            pLs = [psS.tile([128, c.CH], F32, name=f"pL{w}", tag=f"pL{w}")
                   for w in range(c.WCH)]

            # ---------------- phase 1: in-proj, conv, fwd+bwd scans ------
            with ExitStack() as p1:
                xt_pool = p1.enter_context(tc.tile_pool(name="xt", bufs=1))
                xt_t = [xt_pool.tile([128, c.Tw], MD, name=f"xt{k}", tag=f"xt{k}")
                        for k in range(c.KG)]
                for k in range(c.KG):
                    nc.sync.dma_start(xt_t[k][:], xt[k * 128:(k + 1) * 128, :])

                win = p1.enter_context(tc.tile_pool(name="win", bufs=2))
                pch = p1.enter_context(tc.tile_pool(name="pch", bufs=2))
                scr = p1.enter_context(tc.tile_pool(name="scr", bufs=2))
                ps1 = p1.enter_context(tc.tile_pool(name="ps1", bufs=2,
                                                    space="PSUM"))
                ptp = p1.enter_context(tc.tile_pool(name="ptp", bufs=2))
                Twin = c.WCH * c.CH

                def chains(g):
                    # phase-3 channel reductions for group g, issued one
                    # group late as PE filler between in-proj groups
                    pt = ptp.tile([128, Twin], MD, tag="pt")
                    nc.scalar.activation(pt[:], rd(f_t[g][:, c.W:c.W + Twin]),
                                         AF.Exp)
                    pft = ptp.tile([128, Twin], MD, tag="pf")
                    nc.vector.tensor_tensor(pft[:], rd(pt[:]),
                                            rd(f_t[g][:, c.W:c.W + Twin]),
                                            OP.mult)
                    st, sp = (g == 0), (g == c.HG - 1)
                    for w in range(c.WCH):
                        sl = slice(c.W + w * c.CH, c.W + (w + 1) * c.CH)
                        slw = slice(w * c.CH, (w + 1) * c.CH)
                        nc.tensor.matmul(pRs[w][64:65, :], wmg_t[g][:],
                                         f_t[g][:, sl], start=st, stop=sp)
                        nc.tensor.matmul(pLs[w][:], wslot_t[g][:],
                                         f_t[g][:, sl], start=st, stop=sp)
                        nc.tensor.matmul(pRs[w][0:1, :], ones_t[:],
                                         pt[:, slw], start=st, stop=sp)
                        nc.tensor.matmul(pRs[w][32:33, :], ones_t[:],
                                         pft[:, slw], start=st, stop=sp)

                for g in range(c.HG):
                    KW = c.KG * 128
                    wa = win.tile([128, KW], MD, tag="wa")
                    nc.gpsimd.dma_start(wa[:], w_in[:, g * KW:(g + 1) * KW])
                    wd = win.tile([128, KW], MD, tag="wd")
                    off = c.HG * KW
                    nc.gpsimd.dma_start(wd[:], w_in[:, off + g * KW:
                                                    off + (g + 1) * KW])

                    x1 = pch.tile([128, c.Tw], F32, tag="x1")
                    for n, (cs, csz) in enumerate(c.chunks):
                        sl = slice(cs, cs + csz)
                        pa = ps1.tile([128, c.CH], F32, tag="pa")
                        pdt = ps1.tile([128, c.CH], F32, tag="pdt")
                        for k in range(c.KG):
                            nc.tensor.matmul(
                                pa[:, 0:csz], wa[:, k * 128:(k + 1) * 128],
                                xt_t[k][:, sl],
                                start=(k == 0), stop=(k == c.KG - 1))
                        for k in range(c.KG):
                            nc.tensor.matmul(
                                pdt[:, 0:csz], wd[:, k * 128:(k + 1) * 128],
                                xt_t[k][:, sl],
                                start=(k == 0), stop=(k == c.KG - 1))
                        sdt = scr.tile([128, c.CH], F32, tag="sdt")
                        nc.scalar.activation(sdt[:, 0:csz], pdt[:, 0:csz],
                                             AF.Silu, bias=chpc("sbias", g))
                        if n == 0:
                            # zero the left-pad region so the causal conv
                            # sees x_prev=0 at the true sequence start and
                            # u=0 in the pad (silu(0)=0); right-pad garbage
                            # only reaches the halo of f_t, which the bwd
                            # strand re-masks before use
                            nc.gpsimd.tensor_tensor(sdt[:, 0:csz],
                                                    sdt[:, 0:csz],
                                                    mb[:, sl], OP.mult)
                        # x1 = (a + b_a) * silu(dt + sbias)
                        nc.vector.scalar_tensor_tensor(
                            x1[:, sl], pa[:, 0:csz], chpc("ba", g),
                            sdt[:, 0:csz], OP.add, OP.mult)

                    # causal depthwise conv k=2 + silu
                    tmp = pch.tile([128, c.Tw], F32, tag="ta")
                    nc.gpsimd.memset(tmp[:, 0:1], 0.0)
                    nc.vector.tensor_scalar(tmp[:, 1:c.Tw], x1[:, 0:c.Tw - 1],
                                            chpc("k0", g), None, OP.mult)
                    ypre = pch.tile([128, c.Tw], F32, tag="tb")
                    nc.vector.scalar_tensor_tensor(
                        ypre[:], x1[:], chpc("k1", g), tmp[:], OP.mult, OP.add)
                    ysl = pch.tile([128, c.Tw], F32, tag="x1")
                    nc.scalar.activation(ysl[:], ypre[:], AF.Silu)
                    u = pch.tile([128, c.Tw], F32, tag="ta")
                    nc.vector.tensor_scalar(u[:], ysl[:], chpc("omdf", g),
                                            None, OP.mult)

                    # fwd EMA scan over the full work range in one op
                    dfb = chpc("df", g).broadcast_to([128, c.Tw])
                    nc.vector.tensor_tensor_scan(
                        f_t[g][:], dfb, u[:], 0.0, OP.mult, OP.add)

                    # bwd strand for this group, interleaved so the PE
                    # stays busy on the next group's in-proj meanwhile
                    Lw = c.Tw - c.W
                    d1 = p2s.tile([128, Lw], F32, tag="d1")
                    nc.vector.scalar_tensor_tensor(
                        d1[:], rd(f_t[g][:, c.W:c.Tw]), chpc("omdb", g),
                        mb[:, c.W:c.Tw], OP.mult, OP.mult)
                    dbb = chpc("db", g).broadcast_to([128, Lw])
                    nc.vector.tensor_tensor_scan(
                        gb_t[g][:, ::-1], dbb, d1[:, ::-1], 0.0,
                        OP.mult, OP.add)

                    if g > 0:
                        chains(g - 1)
                if True:
                    chains(c.HG - 1)

            # ------------- phase 3: memory, fusion, out ------
            with ExitStack() as p2:
                p3 = p2.enter_context(tc.tile_pool(name="p3", bufs=2))
                pb1 = p2.enter_context(tc.tile_pool(name="pb1", bufs=2))
                wpool = p2.enter_context(tc.tile_pool(name="wp", bufs=2))
                wo_pool = p2.enter_context(tc.tile_pool(name="wo", bufs=1))
                row = p2.enter_context(tc.tile_pool(name="row", bufs=2))
                psM = p2.enter_context(tc.tile_pool(name="psM", bufs=2,
                                                    space="PSUM"))
                psO = p2.enter_context(tc.tile_pool(name="psO", bufs=2,
                                                    space="PSUM"))

                # out-proj weights, loaded once (k-major slabs)
                wo_t = [wo_pool.tile([128, c.O], MD, name=f"wo{k}", tag=f"wo{k}")
                        for k in range(c.HG)]
                for k in range(c.HG):
                    nc.sync.dma_start(wo_t[k][:], w_out[k * 128:(k + 1) * 128, :])

                # stage B1: gate math + broadcasts for both chunks (the
                # gpsimd broadcasts land before any fusion work so chunk 1
                # never queues behind chunk 0's fusion)
                ABSs, E2s = [], []
                Es, pZss = [], []
                for w in range(c.WCH):
                    pL = pLs[w]
                    E = p3.tile([128, c.CH], MD, tag="E")
                    nc.scalar.activation(E[:], pL[:], AF.Exp, bias=bslot_t[:])
                    # reuse row 0 of the (now dead) slot-logit bank for Zs
                    pZs = pL[0:1, :]
                    nc.tensor.matmul(pZs, ones_t[:], E[:],
                                     start=True, stop=True)
                    Es.append(E)
                    pZss.append(pZs)
                for w in range(c.WCH):
                    pR = pRs[w]
                    pZ, pG, pM = pR[0:1, :], pR[32:33, :], pR[64:65, :]
                    E, pZs = Es[w], pZss[w]

                    # per-token gate scalars ([1, CH] rows); A/B/s2 are
                    # packed into one row so a single partition_broadcast
                    # fans all three out
                    Zr = row.tile([1, c.CH], F32, tag="Zr")
                    nc.vector.reciprocal_approx_fast(Zr[:], pZ)
                    lnZ = row.tile([1, c.CH], F32, tag="lnZ")
                    nc.scalar.activation(lnZ[:], pZ, AF.Ln)
                    gz = row.tile([1, c.CH], F32, tag="gz")
                    nc.vector.tensor_tensor(gz[:], pG, Zr[:], OP.mult)
                    ent = row.tile([1, c.CH], F32, tag="ent")
                    nc.vector.tensor_tensor(ent[:], lnZ[:], gz[:], OP.subtract)
                    gate = row.tile([1, c.CH], F32, tag="gate")
                    nc.scalar.activation(gate[:], ent[:], AF.Sigmoid,
                                         scale=scc(SC_SW), bias=scc(SC_SB))
                    mg = row.tile([1, c.CH], F32, tag="mg")
                    nc.scalar.activation(mg[:], pM, AF.Sigmoid,
                                         bias=scc(SC_BMG))
                    Zsr = row.tile([1, c.CH], F32, tag="Zsr")
                    nc.vector.reciprocal_approx_fast(Zsr[:], pZs)
                    # s2 broadcasts first: it alone gates E2 and the
                    # memory-read matmuls; A/B follow for the fusion stage
                    Rs2 = row.tile([1, c.CH], F32, tag="Rs2")
                    nc.vector.scalar_tensor_tensor(
                        Rs2[:], mg[:], scc(SC_F2), Zsr[:], OP.mult, OP.mult)
                    S2B = pb1.tile([128, c.CH], F32, tag="S2B")
                    nc.gpsimd.partition_broadcast(S2B[:], Rs2[:])
                    E2 = p3.tile([128, c.CH], MD, tag="E2")
                    nc.vector.tensor_tensor(E2[:], rd(E[:]), S2B[:], OP.mult)

                    R = row.tile([1, 2 * c.CH], F32, tag="R")
                    nc.vector.tensor_scalar(R[:, 0:c.CH], gate[:], scc(SC_F1),
                                            scc(SC_F0), OP.mult, OP.add)
                    nc.vector.tensor_scalar(R[:, c.CH:2 * c.CH], gate[:],
                                            scc(SC_NF1), None, OP.mult)
                    ABS = pb1.tile([128, 2 * c.CH], F32, tag="ABS")
                    nc.gpsimd.partition_broadcast(ABS[:], R[:])
                    ABSs.append(ABS)
                    E2s.append(E2)

                # stage B2: memory read + fusion + out-proj per chunk
                for w in range(c.WCH):
                    sl = slice(c.W + w * c.CH, c.W + (w + 1) * c.CH)
                    slg = slice(w * c.CH, (w + 1) * c.CH)
                    ABS, E2 = ABSs[w], E2s[w]
                    AB = ABS[:, 0:c.CH]
                    BB = ABS[:, c.CH:2 * c.CH]

                    w_t = []
                    for g in range(c.HG):
                        pm = psM.tile([128, c.CH], F32, tag="pm")
                        nc.tensor.matmul(
                            pm[:], memb_t[:, g * 128:(g + 1) * 128],
                            E2[:], start=True, stop=True)
                        t1 = p3.tile([128, c.CH], F32, tag="t1")
                        nc.vector.tensor_tensor(t1[:], rd(f_t[g][:, sl]),
                                                AB, OP.mult)
                        t2 = p3.tile([128, c.CH], F32, tag="t2")
                        nc.gpsimd.tensor_tensor(t2[:], gb_t[g][:, slg], BB,
                                                OP.mult)
                        t3 = p3.tile([128, c.CH], F32, tag="t3")
                        nc.vector.tensor_tensor(t3[:], t1[:], t2[:], OP.add)
                        wt = wpool.tile([128, c.CH], MD, name=f"w{g}", tag=f"w{g}")
                        nc.vector.tensor_tensor(wt[:], t3[:], pm[:], OP.add)
                        w_t.append(wt)

                    # out-proj
                    for m in range(c.OG):
                        po = psO.tile([128, c.CH], F32, tag="po")
                        for k in range(c.HG):
                            nc.tensor.matmul(
                                po[:], wo_t[k][:, m * 128:(m + 1) * 128],
                                w_t[k][:],
                                start=(k == 0), stop=(k == c.HG - 1))
                        ob = p3.tile([128, c.CH], F32, tag="ob")
                        nc.scalar.activation(ob[:], po[:], AF.Identity,
                                             bias=bout_t[:, m:m + 1])
                        nc.sync.dma_start(out_d[m * 128:(m + 1) * 128, slg], ob[:])

    nc.compile()
    return nc


_PROG_CACHE = {}


def _get_prog(cfg: Cfg):
    key = (cfg.DI, cfg.H, cfg.O, cfg.S, cfg.T, cfg.W, cfg.CH, str(cfg.mm_dtype))
    if key not in _PROG_CACHE:
        _PROG_CACHE[key] = build_program(cfg)
    return _PROG_CACHE[key]


def make_in_maps(cfg, x, W_in, b_in, dt_bias_fwd, conv_k, decay_fwd, decay_bwd,
                 memory, mem_decay, W_mem_gate, b_mem_gate, W_slot, b_slot,
                 W_slot_bwd, b_slot_bwd, fusion_weight, scaler_w, scaler_b,
                 W_out, b_out):
    c = cfg
    x = np.asarray(x)
    B, T, DI = x.shape
    f32 = np.float32
    from concourse.dt import dt as _dt
    mmnp = _dt.np(c.mm_dtype)

    def md(a):
        return np.ascontiguousarray(np.asarray(a, f32).astype(mmnp))

    def sig(v):
        return 1.0 / (1.0 + np.exp(-np.asarray(v, np.float64)))

    def col(v):  # [H] -> [128, HG] column blocks
        return np.ascontiguousarray(np.asarray(v, f32).reshape(c.HG, 128).T)

    df = sig(decay_fwd)
    db = sig(decay_bwd)
    chp = np.concatenate([
        col(conv_k[:, 0]), col(conv_k[:, 1]),
        col((1.0 - df)), col(df),
        col((1.0 - db)), col(db),
        col(np.asarray(b_in)[c.H:] + np.asarray(dt_bias_fwd)),
        col(np.asarray(b_in)[:c.H]),
    ], axis=1).astype(f32)
    bout = np.ascontiguousarray(np.asarray(b_out, f32).reshape(c.OG, 128).T)
    bslot = np.asarray(b_slot_bwd, f32).reshape(128, 1)
    scv = np.zeros((1, 8), f32)
    scv[0, SC_F1] = fusion_weight[1]
    scv[0, SC_F0] = fusion_weight[0]
    scv[0, SC_NF1] = -fusion_weight[1]
    scv[0, SC_F2] = fusion_weight[2]
    scv[0, SC_SW] = scaler_w[0]
    scv[0, SC_SB] = scaler_b[0]
    scv[0, SC_BMG] = b_mem_gate[0]
    mem_bank = (np.asarray(memory) * sig(mem_decay)[:, None]).astype(f32)

    # prepack w_in: [p, half, g, k, m] flattened so each group's k-tiles
    # are one contiguous DMA
    W = np.asarray(W_in, f32).reshape(c.KG, 128, 2, c.HG, 128)
    W_p = np.ascontiguousarray(W.transpose(1, 2, 3, 0, 4).reshape(128, -1))
    shared = {
        "w_in": md(W_p),
        "w_out": md(W_out),
        "w_slot": md(W_slot_bwd),
        "w_mg": md(W_mem_gate),
        "mem_bank": md(mem_bank),
        "ones": md(np.ones((128, 1), f32)),
        "chp": chp, "bout": bout, "bslot": bslot, "sc": scv,
    }
    in_maps = []
    for core in range(8):
        b, j = divmod(core, 2)
        start = j * c.Tout - c.W
        gs, ge = max(0, start), min(T, start + c.Tw)
        xtc = np.zeros((c.DI, c.Tw), mmnp)
        xtc[:, gs - start:ge - start] = x[b, gs:ge, :].T.astype(mmnp)
        mask = np.zeros((1, c.Tw), f32)
        mask[0, gs - start:ge - start] = 1.0
        m = dict(shared)
        m["xt"] = xtc
        m["mask"] = mask
        in_maps.append(m)
    return in_maps


def run(cfg, inputs, trace=False, tmpdir=None):
    nc = _get_prog(cfg)
    in_maps = make_in_maps(cfg, **inputs)
    res = run_bass_kernel_spmd(nc, in_maps, core_ids=list(range(8)),
                               trace=trace, tmpdir=tmpdir)
    B, T = np.asarray(inputs["x"]).shape[0], np.asarray(inputs["x"]).shape[1]
    out = np.empty((B, T, cfg.O), np.float32)
    for core in range(8):
        b, j = divmod(core, 2)
        out[b, j * cfg.Tout:(j + 1) * cfg.Tout, :] = res.results[core]["outT"].T
    return out, res


def kernel(**inputs):
    cfg = Cfg()
    out, _ = run(cfg, inputs)
    return out
